# revision 8
# baseline (speedup 1.0000x reference)
"""GCN layer (BGRL-style) on 8 Trainium2 NeuronCores.

Math: the reference computes
  log_softmax(relu((A_hat @ (X*norm_src)) @ W_conv * norm_dst + b) @ W2 + b2).
Aggregation is linear and in_feats > hidden, so each core first computes
h = (X*norm_src) @ W_conv for ALL nodes (redundantly, avoiding collectives)
into DRAM h-tables (bf16, 512B rows), then aggregates h[src] per
destination block — halving the per-edge gather traffic vs gathering raw
features.

The h-tables are PARTITION-MAJOR: node n = nb*128 + p lives at table row
p*392 + nb, split into h_lo (p < 64) and h_hi (p >= 64) so dma_gather's
int16 indices stay in range (25088 rows each). Phase-1 stores are then
4KB-contiguous per partition (one store per table per 8-block batch), on
both HWDGE rings (loads via nc.sync/SP, stores via nc.scalar/ACT).

Sharding: dst nodes are greedily assigned to 8*49 groups of <=128 slots,
jointly balancing each group's lo- and hi-edge counts; the host unpermutes
output rows at the end.

Per 128-dst block the core, fully on-chip in one pass:
  - fetches h[src] rows with TWO dma_gather instructions (SWDGE ucode,
    ~1us fixed + 0.34ns/row) from h_lo/h_hi, landing [128, T, 256] tiles
    in exactly the (lane=i%128, tile=i//128) edge order,
  - segment-sums TRANSPOSED via one-hot S matmuls (gathered h-chunks as
    lhsT, S as rhs) -> xT [h x d] in PSUM, so the whole downstream chain
    needs no transposes: a free-dim broadcast multiply by norm_dst,
    relu+bias (per-partition, h on partitions), W2 matmuls, bias
    outer-products via K=1 matmuls, and log_softmax, streaming fp32 logits
    out per block.
All 8 cores run one SPMD program; edge partitions are padded to uniform
tiles-per-block counts T_LO/T_HI (pad lanes get idx 0 + sentinel dst 255,
whose S column is all-zero).
"""

import numpy as np

N = 50000
F = 512
H = 256
C = 64
P = 8
NB = 49                  # dst blocks per core
NG = P * NB              # 392 dst groups, <=128 nodes each
NPC = NB * 128           # 6272 output rows per core (incl. pad slots)
NBLK = 392               # node blocks for phase 1 (392*128 = 50176 >= N)
NPAD = NBLK * 128
NHALF = 64 * NBLK        # 25088 rows per h table (partition-major halves)
GB = 8                   # node blocks per phase-1 batch
NGRP1 = NBLK // GB       # 49 phase-1 batches
NQ = 4                   # SWDGE queues

_cache = {}
_trace = False          # set by test harness for profiled runs
_trace_tmpdir = None
_last_results = None


def _build_program(T_LO, T_HI, bench_R=0):
    import concourse.mybir as mybir
    import concourse.tile as tile
    from concourse import bacc, library_config
    from concourse.tile_rust import add_dep_helper

    dt = mybir.dt
    T_T = T_LO + T_HI       # edge tiles per dst block
    NT = NB * T_T           # edge tiles per core

    nc = bacc.Bacc("TRN2", target_bir_lowering=False, debug=False,
                   num_devices=P, num_swdge_queues=NQ,
                   dynamic_dma_scratch_size=32768)

    featT_d = nc.dram_tensor("featT", [128, NBLK * 512], dt.bfloat16,
                             kind="ExternalInput")
    h_lo_d = nc.dram_tensor("h_lo", [NHALF, H], dt.bfloat16, kind="Internal")
    h_hi_d = nc.dram_tensor("h_hi", [NHALF, H], dt.bfloat16, kind="Internal")
    ixlo_d = nc.dram_tensor("ixlo", [128, NB * T_LO * 8], dt.int16,
                            kind="ExternalInput")
    ixhi_d = nc.dram_tensor("ixhi", [128, NB * T_HI * 8], dt.int16,
                            kind="ExternalInput")
    dstloc_d = nc.dram_tensor("dstloc", [128, NT], dt.bfloat16,
                              kind="ExternalInput")
    # normdst materialized across partitions (DVE lanes cannot broadcast
    # along the partition dim): every row identical
    normdst_d = nc.dram_tensor("normdst", [128, NB * 128], dt.bfloat16,
                               kind="ExternalInput")
    iota_d = nc.dram_tensor("iota", [128, 128], dt.bfloat16, kind="ExternalInput")
    wconv_d = nc.dram_tensor("wconv", [128, 4 * H], dt.bfloat16,
                             kind="ExternalInput")
    w2_d = nc.dram_tensor("w2", [128, 2 * C], dt.bfloat16, kind="ExternalInput")
    ones_d = nc.dram_tensor("ones1", [1, 128], dt.bfloat16, kind="ExternalInput")
    # bconv as [128, 2] (h on partitions, one col per h-half)
    bconv_d = nc.dram_tensor("bconv", [128, 2], dt.float32, kind="ExternalInput")
    b2_d = nc.dram_tensor("b2r", [1, C], dt.bfloat16, kind="ExternalInput")
    out_d = nc.dram_tensor("out", [NPC, C], dt.float32, kind="ExternalOutput")

    h_lo_pm = h_lo_d[:].rearrange("(p n) c -> p n c", p=64)
    h_hi_pm = h_hi_d[:].rearrange("(p n) c -> p n c", p=64)

    with tile.TileContext(nc) as tc:
        with (
            tc.tile_pool(name="const", bufs=1) as cpool,
            tc.tile_pool(name="x1", bufs=2) as xpool,
            tc.tile_pool(name="h1", bufs=2) as hpool,
            tc.tile_pool(name="work", bufs=3) as wpool,
            tc.tile_pool(name="gath", bufs=4) as gpool,
            tc.tile_pool(name="psA", bufs=4, space="PSUM") as ppool,
            tc.tile_pool(name="psB", bufs=2, space="PSUM") as ppool1,
        ):
            lib = nc.gpsimd.load_library(library_config.mlp)

            # --- constants / metadata, loaded once ---
            iota_t = cpool.tile([128, 128], dt.bfloat16, tag="iota")
            nc.sync.dma_start(iota_t[:], iota_d[:])
            wconv_t = cpool.tile([128, 4 * H], dt.bfloat16, tag="wconv")
            nc.sync.dma_start(wconv_t[:], wconv_d[:])
            w2_t = cpool.tile([128, 2 * C], dt.bfloat16, tag="w2")
            nc.sync.dma_start(w2_t[:], w2_d[:])
            ones_t = cpool.tile([1, 128], dt.bfloat16, tag="ones")
            nc.sync.dma_start(ones_t[:], ones_d[:])
            bconv_t = cpool.tile([128, 2], dt.float32, tag="bconv")
            nc.sync.dma_start(bconv_t[:], bconv_d[:])
            b2_t = cpool.tile([1, C], dt.bfloat16, tag="b2")
            nc.sync.dma_start(b2_t[:], b2_d[:])
            ixlo_t = cpool.tile([128, NB * T_LO * 8], dt.int16, tag="ixlo")
            nc.sync.dma_start(ixlo_t[:], ixlo_d[:])
            ixhi_t = cpool.tile([128, NB * T_HI * 8], dt.int16, tag="ixhi")
            nc.sync.dma_start(ixhi_t[:], ixhi_d[:])
            dstloc_t = cpool.tile([128, NT], dt.bfloat16, tag="dstloc")
            nc.sync.dma_start(dstloc_t[:], dstloc_d[:])
            normdst_t = cpool.tile([128, NB * 128], dt.bfloat16, tag="normdst")
            nc.sync.dma_start(normdst_t[:], normdst_d[:])

            iota_rep = iota_t[:].rearrange("p (o n) -> p o n", o=1).broadcast_to(
                [128, T_T, 128]
            )

            def body():
                # --- phase 1: h = Xn @ W_conv for all nodes -> DRAM ---
                for gi in range(NGRP1):
                    xt = xpool.tile([128, GB, 512], dt.bfloat16, tag="xt")
                    nc.sync.dma_start(
                        xt[:], featT_d[:, gi * GB * 512:(gi + 1) * GB * 512]
                        .rearrange("p (t f) -> p t f", t=GB))
                    hsb = hpool.tile([128, GB, H], dt.bfloat16, tag="hsb")
                    for t in range(GB):
                        hps = ppool.tile([128, H], dt.float32, tag="hps")
                        for c in range(4):
                            nc.tensor.matmul(
                                hps[:],
                                xt[:, t, c * 128:(c + 1) * 128],
                                wconv_t[:, c * H:(c + 1) * H],
                                start=(c == 0), stop=(c == 3),
                            )
                        nc.vector.tensor_copy(hsb[:, t, :], hps[:])
                    # node (gi*8+t)*128 + p -> table row (p%64)*392 + gi*8+t
                    nc.scalar.dma_start(
                        h_lo_pm[:, gi * GB:(gi + 1) * GB, :], hsb[0:64])
                    nc.scalar.dma_start(
                        h_hi_pm[:, gi * GB:(gi + 1) * GB, :], hsb[64:128])

                # --- phase 2: per dst block gather + aggregate + head ---
                for b in range(NB):
                    # S one-hot: S[p, t*128+j] = (dstloc[p, b*T_T+t] == j)
                    S = wpool.tile([128, T_T, 128], dt.bfloat16, tag="S",
                                   bufs=2)
                    nc.vector.tensor_tensor(
                        S[:],
                        iota_rep,
                        dstloc_t[:, b * T_T:(b + 1) * T_T].broadcast_to(
                            [128, T_T, 128]
                        ),
                        op=mybir.AluOpType.is_equal,
                    )
                    g = gpool.tile([128, T_T, H], dt.bfloat16, tag="g")
                    glo = nc.gpsimd.dma_gather(
                        g[:, 0:T_LO, :], h_lo_d[:],
                        ixlo_t[:, b * T_LO * 8:(b + 1) * T_LO * 8],
                        T_LO * 128, T_LO * 128, H,
                        queue_num=(2 * b) % NQ, single_packet=False)
                    add_dep_helper(glo.ins, lib.ins, reason="lib first")
                    ghi = nc.gpsimd.dma_gather(
                        g[:, T_LO:T_T, :], h_hi_d[:],
                        ixhi_t[:, b * T_HI * 8:(b + 1) * T_HI * 8],
                        T_HI * 128, T_HI * 128, H,
                        queue_num=(2 * b + 1) % NQ, single_packet=False)
                    add_dep_helper(ghi.ins, lib.ins, reason="lib first")
                    # xT[half] [128h x 128d] += g[:,t,half].T @ S[:,t,:]
                    xtp = ppool1.tile([128, H], dt.float32, tag="xtp")
                    for t in range(T_T):
                        for half in range(2):
                            nc.tensor.matmul(
                                xtp[:, half * 128:(half + 1) * 128],
                                g[:, t, half * 128:(half + 1) * 128],
                                S[:, t, :],
                                start=(t == 0), stop=(t == T_T - 1),
                            )
                    # x = relu(xT * norm_dst[d] + b_conv[h]); norm_dst along
                    # free dim (d), bias per partition (h)
                    xn = wpool.tile([128, H], dt.float32, tag="xn")
                    nc.vector.tensor_tensor(
                        xn[:].rearrange("p (o n) -> p o n", o=2),
                        xtp[:].rearrange("p (o n) -> p o n", o=2),
                        normdst_t[:, b * 128:(b + 1) * 128]
                        .rearrange("p (o n) -> p o n", o=1)
                        .broadcast_to([128, 2, 128]),
                        op=mybir.AluOpType.mult,
                    )
                    xts = wpool.tile([128, H], dt.bfloat16, tag="xts")
                    for half in range(2):
                        # relu(xn + bconv) fused on DVE (add then max 0)
                        nc.vector.tensor_scalar(
                            xts[:, half * 128:(half + 1) * 128],
                            xn[:, half * 128:(half + 1) * 128],
                            bconv_t[:, half:half + 1], 0.0,
                            op0=mybir.AluOpType.add,
                            op1=mybir.AluOpType.max,
                        )
                    # logits [128d x 64] = sum_half xts[half].T @ w2[half] + b2
                    lps = ppool1.tile([128, C], dt.float32, tag="lps")
                    for half in range(2):
                        nc.tensor.matmul(
                            lps[:], xts[:, half * 128:(half + 1) * 128],
                            w2_t[:, half * C:(half + 1) * C],
                            start=(half == 0), stop=False,
                        )
                    nc.tensor.matmul(lps[:], ones_t[:], b2_t[:],
                                     start=False, stop=True)
                    # log_softmax along classes
                    mneg = wpool.tile([128, 1], dt.float32, tag="mneg")
                    nc.vector.reduce_max(mneg[:], lps[:],
                                         axis=mybir.AxisListType.X, negate=True)
                    esc = wpool.tile([128, C], dt.float32, tag="esc")
                    ssum = wpool.tile([128, 1], dt.float32, tag="ssum")
                    nc.scalar.activation(
                        esc[:], lps[:], mybir.ActivationFunctionType.Exp,
                        bias=mneg[:], accum_out=ssum[:],
                    )
                    lse = wpool.tile([128, 1], dt.float32, tag="lse")
                    nc.scalar.activation(lse[:], ssum[:],
                                         mybir.ActivationFunctionType.Ln)
                    shift = wpool.tile([128, 1], dt.float32, tag="shift")
                    nc.vector.tensor_tensor(shift[:], mneg[:], lse[:],
                                            op=mybir.AluOpType.subtract)
                    osb = wpool.tile([128, C], dt.float32, tag="osb")
                    nc.vector.tensor_scalar_add(osb[:], lps[:], shift[:])
                    nc.sync.dma_start(out_d[b * 128:(b + 1) * 128, :], osb[:])

            if bench_R:
                with tc.For_i(0, bench_R, 1):
                    body()
            else:
                body()

    nc.compile()
    return nc


def _balance_groups(deg_lo, deg_hi):
    """Greedily assign nodes to NG groups (<=128 each), jointly balancing
    lo- and hi-edge sums: each node goes to the non-full group minimizing
    max(lo_sum+dlo, hi_sum+dhi).

    Returns (grp_of, slot_of) int32 arrays of length N."""
    order = np.argsort(-(deg_lo + deg_hi), kind="stable")
    lo_s = np.zeros(NG, np.float64)
    hi_s = np.zeros(NG, np.float64)
    sizes = np.zeros(NG, np.int64)
    grp_of = np.empty(N, np.int32)
    slot_of = np.empty(N, np.int32)
    full = np.zeros(NG, bool)
    BIG = 1e18
    for n in order.tolist():
        cost = np.maximum(lo_s + deg_lo[n], hi_s + deg_hi[n])
        cost[full] = BIG
        g = int(np.argmin(cost))
        grp_of[n] = g
        slot_of[n] = sizes[g]
        sizes[g] += 1
        if sizes[g] == 128:
            full[g] = True
        lo_s[g] += deg_lo[n]
        hi_s[g] += deg_hi[n]
    return grp_of, slot_of


def _pack_idx(vals):
    """[T*128] int16 -> [128, T*8] device idx layout.

    dma_gather reads idx i from [i % 16, i // 16] of a [16, NI/16] block,
    replicated across the 8 gpsimd core groups."""
    a = vals.reshape(-1, 16).T          # [16, NI/16]
    return np.tile(a, (8, 1))


def _prep(features, W_conv, b_conv, W2, b2, src, dst):
    import ml_dtypes
    bf16 = ml_dtypes.bfloat16

    src = np.asarray(src).astype(np.int64)
    dst = np.asarray(dst).astype(np.int64)
    deg_out = np.bincount(src, minlength=N).astype(np.float32)
    deg_in = np.bincount(dst, minlength=N).astype(np.float32)
    norm_src = 1.0 / np.sqrt(deg_out)
    norm_dst = 1.0 / np.sqrt(deg_in)

    # partition-major h-table row of node s: (s%128 % 64)*392 + s//128,
    # in h_lo for s%128 < 64 else h_hi
    s_p = src % 128
    s_row = (s_p % 64) * NBLK + src // 128
    elo = s_p < 64

    # per-dst lo/hi in-degrees for the 2D balance
    dlo = np.bincount(dst[elo], minlength=N).astype(np.float32)
    dhi = deg_in - dlo
    grp_of, slot_of = _balance_groups(dlo, dhi)

    # normalized features, transposed + blocked for phase 1:
    # featT[p, (nb*4 + c)*128 + j] = Xn[nb*128 + j, c*128 + p]
    feat_n = (np.asarray(features, np.float32) * norm_src[:, None]).astype(bf16)
    xp = np.zeros((NPAD, F), bf16)
    xp[:N] = feat_n
    featT = np.ascontiguousarray(
        xp.reshape(NBLK, 128, 4, 128).transpose(3, 0, 2, 1)
    ).reshape(128, NBLK * 512)

    # edges -> (group, lo/hi, tile, lane)
    eg = grp_of[dst]
    cnt_lo = np.bincount(eg[elo], minlength=NG)
    cnt_hi = np.bincount(eg[~elo], minlength=NG)
    T_LO = int(np.ceil(cnt_lo.max() / 128))
    T_HI = int(np.ceil(cnt_hi.max() / 128))

    dloc = slot_of[dst].astype(np.float32)

    def layout(mask, cnts, L):
        r_e = s_row[mask]
        g_e = eg[mask]
        d_e = dloc[mask]
        # sort by (group, table row) so each tile's rows ascend in the table
        order = np.lexsort((r_e, g_e))
        g_s = g_e[order]
        starts = np.zeros(NG + 1, np.int64)
        np.cumsum(cnts, out=starts[1:])
        pos = np.arange(len(g_s)) - starts[g_s]
        slot = g_s * L + pos
        idx_pad = np.zeros(NG * L, np.int16)
        dl_pad = np.full(NG * L, 255.0, np.float32)
        idx_pad[slot] = r_e[order].astype(np.int16)
        dl_pad[slot] = d_e[order]
        return idx_pad.reshape(NG, L), dl_pad.reshape(NG, L)

    idx_lo, dl_lo = layout(elo, cnt_lo, T_LO * 128)
    idx_hi, dl_hi = layout(~elo, cnt_hi, T_HI * 128)

    # normdst per core: [NB*128] values in permuted slot order, pad 1.0
    nd = np.ones(NG * 128, np.float32)
    nd[grp_of * 128 + slot_of] = norm_dst

    iota = np.broadcast_to(np.arange(128, dtype=np.float32), (128, 128)).astype(bf16)
    wconv = np.ascontiguousarray(
        np.asarray(W_conv, np.float32).reshape(4, 128, H).transpose(1, 0, 2)
    ).reshape(128, 4 * H).astype(bf16)
    w2r = np.ascontiguousarray(
        np.asarray(W2, np.float32).reshape(2, 128, C).transpose(1, 0, 2)
    ).reshape(128, 2 * C).astype(bf16)

    in_maps = []
    for c in range(P):
        gsl = slice(c * NB, (c + 1) * NB)
        # dstloc per core: [128, NT] where tile column (b*T_T + t), lane p
        # holds the slot of edge (tile t of block b, lane p); lo tiles then
        # hi tiles within each block
        dl_c = np.concatenate(
            [dl_lo[gsl].reshape(NB, T_LO, 128),
             dl_hi[gsl].reshape(NB, T_HI, 128)], axis=1
        )  # [NB, T_T, 128]
        dstloc = np.ascontiguousarray(
            dl_c.reshape(NB * (T_LO + T_HI), 128).T).astype(bf16)
        ixlo = np.concatenate(
            [_pack_idx(idx_lo[g]) for g in range(c * NB, (c + 1) * NB)],
            axis=1)
        ixhi = np.concatenate(
            [_pack_idx(idx_hi[g]) for g in range(c * NB, (c + 1) * NB)],
            axis=1)
        in_maps.append({
            "featT": featT,
            "ixlo": np.ascontiguousarray(ixlo),
            "ixhi": np.ascontiguousarray(ixhi),
            "dstloc": dstloc,
            "normdst": np.ascontiguousarray(np.broadcast_to(
                nd[c * NPC:(c + 1) * NPC].astype(bf16), (128, NB * 128))),
            "iota": iota,
            "wconv": wconv,
            "w2": w2r,
            "ones1": np.ones((1, 128), np.float32).astype(bf16),
            "bconv": np.asarray(b_conv, np.float32).reshape(2, 128).T.copy(),
            "b2r": np.asarray(b2, np.float32).reshape(1, C).astype(bf16),
        })
    return T_LO, T_HI, grp_of, slot_of, in_maps


def kernel(features, W_conv, b_conv, W2, b2, src, dst):
    from concourse.bass_utils import run_bass_kernel_spmd

    T_LO, T_HI, grp_of, slot_of, in_maps = _prep(
        features, W_conv, b_conv, W2, b2, src, dst)
    key = (T_LO, T_HI)
    if key not in _cache:
        _cache[key] = _build_program(T_LO, T_HI)
    nc = _cache[key]
    res = run_bass_kernel_spmd(nc, in_maps, core_ids=list(range(P)),
                               trace=_trace, tmpdir=_trace_tmpdir)
    global _last_results
    _last_results = res
    rows = np.concatenate([res.results[c]["out"] for c in range(P)], axis=0)
    out = rows[grp_of * 128 + slot_of]
    return out.astype(np.float32)


# revision 10
# speedup vs baseline: 1.5535x; 1.5535x over previous
"""GCN layer (BGRL-style) on 8 Trainium2 NeuronCores.

Math: the reference computes
  log_softmax(relu((A_hat @ (X*norm_src)) @ W_conv * norm_dst + b) @ W2 + b2).
Aggregation is linear and in_feats > hidden, so each core first computes
h = (X*norm_src) @ W_conv for ALL nodes (redundantly, avoiding collectives)
into DRAM h-tables (bf16, 512B rows), then aggregates h[src] per
destination block — halving the per-edge gather traffic vs gathering raw
features.

The h-tables are PARTITION-MAJOR: node n = nb*128 + p lives at table row
p*392 + nb, split into h_lo (p < 64) and h_hi (p >= 64) so dma_gather's
int16 indices stay in range (25088 rows each). Phase-1 stores are then
4KB-contiguous per partition (one store per table per 8-block batch), on
both HWDGE rings (loads via nc.sync/SP, stores via nc.scalar/ACT).

Sharding: dst nodes are greedily assigned to 8*49 groups of <=128 slots,
jointly balancing each group's lo- and hi-edge counts; the host unpermutes
output rows at the end.

Per 128-dst block the core, fully on-chip in one pass:
  - fetches h[src] rows with TWO dma_gather instructions (SWDGE ucode,
    ~1us fixed + 0.34ns/row) from h_lo/h_hi, landing [128, T, 256] tiles
    in exactly the (lane=i%128, tile=i//128) edge order,
  - segment-sums TRANSPOSED via one-hot S matmuls (gathered h-chunks as
    lhsT, S as rhs) -> xT [h x d] in PSUM, so the whole downstream chain
    needs no transposes: a free-dim broadcast multiply by norm_dst,
    relu+bias (per-partition, h on partitions), W2 matmuls, bias
    outer-products via K=1 matmuls, and log_softmax, streaming fp32 logits
    out per block.
All 8 cores run one SPMD program; edge partitions are padded to uniform
tiles-per-block counts T_LO/T_HI (pad lanes get idx 0 + sentinel dst 255,
whose S column is all-zero).
"""

import numpy as np

N = 50000
F = 512
H = 256
C = 64
P = 8
NB = 49                  # dst blocks per core
NG = P * NB              # 392 dst groups, <=128 nodes each
NPC = NB * 128           # 6272 output rows per core (incl. pad slots)
NBLK = 392               # node blocks for phase 1 (392*128 = 50176 >= N)
NPAD = NBLK * 128
NHALF = 64 * NBLK        # 25088 rows per h table (partition-major halves)
GB = 8                   # node blocks per phase-1 batch
NGRP1 = NBLK // GB       # 49 phase-1 batches
NQ = 4                   # SWDGE queues

_cache = {}
_trace = False          # set by test harness for profiled runs
_trace_tmpdir = None
_last_results = None


def _build_program(T_LO, T_HI, bench_R=0):
    import concourse.mybir as mybir
    import concourse.tile as tile
    from concourse import bacc, library_config
    from concourse.tile_rust import add_dep_helper

    dt = mybir.dt
    T_T = T_LO + T_HI       # edge tiles per dst block
    NT = NB * T_T           # edge tiles per core

    nc = bacc.Bacc("TRN2", target_bir_lowering=False, debug=False,
                   num_devices=P, num_swdge_queues=NQ)

    featT_d = nc.dram_tensor("featT", [128, NBLK * 512], dt.bfloat16,
                             kind="ExternalInput")
    h_lo_d = nc.dram_tensor("h_lo", [NHALF, H], dt.bfloat16, kind="Internal")
    h_hi_d = nc.dram_tensor("h_hi", [NHALF, H], dt.bfloat16, kind="Internal")
    ixlo_d = nc.dram_tensor("ixlo", [128, NB * T_LO * 8], dt.int16,
                            kind="ExternalInput")
    ixhi_d = nc.dram_tensor("ixhi", [128, NB * T_HI * 8], dt.int16,
                            kind="ExternalInput")
    dstloc_d = nc.dram_tensor("dstloc", [128, NT], dt.bfloat16,
                              kind="ExternalInput")
    # normdst materialized across partitions (DVE lanes cannot broadcast
    # along the partition dim): every row identical
    normdst_d = nc.dram_tensor("normdst", [128, NB * 128], dt.bfloat16,
                               kind="ExternalInput")
    iota_d = nc.dram_tensor("iota", [128, 128], dt.bfloat16, kind="ExternalInput")
    wconv_d = nc.dram_tensor("wconv", [128, 4 * H], dt.bfloat16,
                             kind="ExternalInput")
    w2_d = nc.dram_tensor("w2", [128, 2 * C], dt.bfloat16, kind="ExternalInput")
    ones_d = nc.dram_tensor("ones1", [1, 128], dt.bfloat16, kind="ExternalInput")
    # bconv as [128, 2] (h on partitions, one col per h-half)
    bconv_d = nc.dram_tensor("bconv", [128, 2], dt.float32, kind="ExternalInput")
    b2_d = nc.dram_tensor("b2r", [1, C], dt.bfloat16, kind="ExternalInput")
    out_d = nc.dram_tensor("out", [NPC, C], dt.float32, kind="ExternalOutput")

    h_lo_pm = h_lo_d[:].rearrange("(p n) c -> p n c", p=64)
    h_hi_pm = h_hi_d[:].rearrange("(p n) c -> p n c", p=64)

    with tile.TileContext(nc) as tc:
        with (
            tc.tile_pool(name="const", bufs=1) as cpool,
            tc.tile_pool(name="x1", bufs=2) as xpool,
            tc.tile_pool(name="h1", bufs=2) as hpool,
            tc.tile_pool(name="work", bufs=3) as wpool,
            tc.tile_pool(name="gath", bufs=4) as gpool,
            tc.tile_pool(name="psA", bufs=4, space="PSUM") as ppool,
            tc.tile_pool(name="psB", bufs=2, space="PSUM") as ppool1,
        ):
            lib = nc.gpsimd.load_library(library_config.mlp)

            # --- constants / metadata, loaded once ---
            iota_t = cpool.tile([128, 128], dt.bfloat16, tag="iota")
            nc.sync.dma_start(iota_t[:], iota_d[:])
            wconv_t = cpool.tile([128, 4 * H], dt.bfloat16, tag="wconv")
            nc.sync.dma_start(wconv_t[:], wconv_d[:])
            w2_t = cpool.tile([128, 2 * C], dt.bfloat16, tag="w2")
            nc.sync.dma_start(w2_t[:], w2_d[:])
            ones_t = cpool.tile([1, 128], dt.bfloat16, tag="ones")
            nc.sync.dma_start(ones_t[:], ones_d[:])
            bconv_t = cpool.tile([128, 2], dt.float32, tag="bconv")
            nc.sync.dma_start(bconv_t[:], bconv_d[:])
            b2_t = cpool.tile([1, C], dt.bfloat16, tag="b2")
            nc.sync.dma_start(b2_t[:], b2_d[:])
            ixlo_t = cpool.tile([128, NB * T_LO * 8], dt.int16, tag="ixlo")
            nc.sync.dma_start(ixlo_t[:], ixlo_d[:])
            ixhi_t = cpool.tile([128, NB * T_HI * 8], dt.int16, tag="ixhi")
            nc.sync.dma_start(ixhi_t[:], ixhi_d[:])
            dstloc_t = cpool.tile([128, NT], dt.bfloat16, tag="dstloc")
            nc.sync.dma_start(dstloc_t[:], dstloc_d[:])
            normdst_t = cpool.tile([128, NB * 128], dt.bfloat16, tag="normdst")
            nc.sync.dma_start(normdst_t[:], normdst_d[:])

            iota_rep = iota_t[:].rearrange("p (o n) -> p o n", o=1).broadcast_to(
                [128, T_T, 128]
            )

            def body():
                # --- phase 1: h = Xn @ W_conv for all nodes -> DRAM ---
                for gi in range(NGRP1):
                    xt = xpool.tile([128, GB, 512], dt.bfloat16, tag="xt")
                    nc.sync.dma_start(
                        xt[:], featT_d[:, gi * GB * 512:(gi + 1) * GB * 512]
                        .rearrange("p (t f) -> p t f", t=GB))
                    hsb = hpool.tile([128, GB, H], dt.bfloat16, tag="hsb")
                    for t in range(GB):
                        hps = ppool.tile([128, H], dt.float32, tag="hps")
                        for c in range(4):
                            nc.tensor.matmul(
                                hps[:],
                                xt[:, t, c * 128:(c + 1) * 128],
                                wconv_t[:, c * H:(c + 1) * H],
                                start=(c == 0), stop=(c == 3),
                            )
                        nc.vector.tensor_copy(hsb[:, t, :], hps[:])
                    # node (gi*8+t)*128 + p -> table row (p%64)*392 + gi*8+t
                    nc.scalar.dma_start(
                        h_lo_pm[:, gi * GB:(gi + 1) * GB, :], hsb[0:64])
                    nc.scalar.dma_start(
                        h_hi_pm[:, gi * GB:(gi + 1) * GB, :], hsb[64:128])

                # --- phase 2: per dst block gather + aggregate + head ---
                for b in range(NB):
                    # S one-hot: S[p, t*128+j] = (dstloc[p, b*T_T+t] == j)
                    S = wpool.tile([128, T_T, 128], dt.bfloat16, tag="S",
                                   bufs=2)
                    nc.vector.tensor_tensor(
                        S[:],
                        iota_rep,
                        dstloc_t[:, b * T_T:(b + 1) * T_T].broadcast_to(
                            [128, T_T, 128]
                        ),
                        op=mybir.AluOpType.is_equal,
                    )
                    g = gpool.tile([128, T_T, H], dt.bfloat16, tag="g")
                    glo = nc.gpsimd.dma_gather(
                        g[:, 0:T_LO, :], h_lo_d[:],
                        ixlo_t[:, b * T_LO * 8:(b + 1) * T_LO * 8],
                        T_LO * 128, T_LO * 128, H,
                        queue_num=(2 * b) % NQ, single_packet=False)
                    add_dep_helper(glo.ins, lib.ins, reason="lib first")
                    ghi = nc.gpsimd.dma_gather(
                        g[:, T_LO:T_T, :], h_hi_d[:],
                        ixhi_t[:, b * T_HI * 8:(b + 1) * T_HI * 8],
                        T_HI * 128, T_HI * 128, H,
                        queue_num=(2 * b + 1) % NQ, single_packet=False)
                    add_dep_helper(ghi.ins, lib.ins, reason="lib first")
                    # xT[half] [128h x 128d] += g[:,t,half].T @ S[:,t,:]
                    xtp = ppool1.tile([128, H], dt.float32, tag="xtp")
                    for t in range(T_T):
                        for half in range(2):
                            nc.tensor.matmul(
                                xtp[:, half * 128:(half + 1) * 128],
                                g[:, t, half * 128:(half + 1) * 128],
                                S[:, t, :],
                                start=(t == 0), stop=(t == T_T - 1),
                            )
                    # x = relu(xT * norm_dst[d] + b_conv[h]); norm_dst along
                    # free dim (d), bias per partition (h)
                    xn = wpool.tile([128, H], dt.float32, tag="xn")
                    nc.vector.tensor_tensor(
                        xn[:].rearrange("p (o n) -> p o n", o=2),
                        xtp[:].rearrange("p (o n) -> p o n", o=2),
                        normdst_t[:, b * 128:(b + 1) * 128]
                        .rearrange("p (o n) -> p o n", o=1)
                        .broadcast_to([128, 2, 128]),
                        op=mybir.AluOpType.mult,
                    )
                    xts = wpool.tile([128, H], dt.bfloat16, tag="xts")
                    for half in range(2):
                        nc.scalar.activation(
                            xts[:, half * 128:(half + 1) * 128],
                            xn[:, half * 128:(half + 1) * 128],
                            mybir.ActivationFunctionType.Relu,
                            bias=bconv_t[:, half:half + 1],
                        )
                    # logits [128d x 64] = sum_half xts[half].T @ w2[half] + b2
                    lps = ppool1.tile([128, C], dt.float32, tag="lps")
                    for half in range(2):
                        nc.tensor.matmul(
                            lps[:], xts[:, half * 128:(half + 1) * 128],
                            w2_t[:, half * C:(half + 1) * C],
                            start=(half == 0), stop=False,
                        )
                    nc.tensor.matmul(lps[:], ones_t[:], b2_t[:],
                                     start=False, stop=True)
                    # log_softmax along classes
                    mneg = wpool.tile([128, 1], dt.float32, tag="mneg")
                    nc.vector.reduce_max(mneg[:], lps[:],
                                         axis=mybir.AxisListType.X, negate=True)
                    esc = wpool.tile([128, C], dt.float32, tag="esc")
                    ssum = wpool.tile([128, 1], dt.float32, tag="ssum")
                    nc.scalar.activation(
                        esc[:], lps[:], mybir.ActivationFunctionType.Exp,
                        bias=mneg[:], accum_out=ssum[:],
                    )
                    lse = wpool.tile([128, 1], dt.float32, tag="lse")
                    nc.scalar.activation(lse[:], ssum[:],
                                         mybir.ActivationFunctionType.Ln)
                    shift = wpool.tile([128, 1], dt.float32, tag="shift")
                    nc.vector.tensor_tensor(shift[:], mneg[:], lse[:],
                                            op=mybir.AluOpType.subtract)
                    osb = wpool.tile([128, C], dt.float32, tag="osb")
                    nc.vector.tensor_scalar_add(osb[:], lps[:], shift[:])
                    nc.sync.dma_start(out_d[b * 128:(b + 1) * 128, :], osb[:])

            if bench_R:
                with tc.For_i(0, bench_R, 1):
                    body()
            else:
                body()

    nc.compile()
    return nc


def _balance_groups(deg_lo, deg_hi):
    """Greedily assign nodes to NG groups (<=128 each), jointly balancing
    lo- and hi-edge sums: each node goes to the non-full group minimizing
    max(lo_sum+dlo, hi_sum+dhi).

    Returns (grp_of, slot_of) int32 arrays of length N."""
    order = np.argsort(-(deg_lo + deg_hi), kind="stable")
    lo_s = np.zeros(NG, np.float64)
    hi_s = np.zeros(NG, np.float64)
    sizes = np.zeros(NG, np.int64)
    grp_of = np.empty(N, np.int32)
    slot_of = np.empty(N, np.int32)
    full = np.zeros(NG, bool)
    BIG = 1e18
    for n in order.tolist():
        cost = np.maximum(lo_s + deg_lo[n], hi_s + deg_hi[n])
        cost[full] = BIG
        g = int(np.argmin(cost))
        grp_of[n] = g
        slot_of[n] = sizes[g]
        sizes[g] += 1
        if sizes[g] == 128:
            full[g] = True
        lo_s[g] += deg_lo[n]
        hi_s[g] += deg_hi[n]
    return grp_of, slot_of


def _pack_idx(vals):
    """[T*128] int16 -> [128, T*8] device idx layout.

    dma_gather reads idx i from [i % 16, i // 16] of a [16, NI/16] block,
    replicated across the 8 gpsimd core groups."""
    a = vals.reshape(-1, 16).T          # [16, NI/16]
    return np.tile(a, (8, 1))


def _prep(features, W_conv, b_conv, W2, b2, src, dst):
    import ml_dtypes
    bf16 = ml_dtypes.bfloat16

    src = np.asarray(src).astype(np.int64)
    dst = np.asarray(dst).astype(np.int64)
    deg_out = np.bincount(src, minlength=N).astype(np.float32)
    deg_in = np.bincount(dst, minlength=N).astype(np.float32)
    norm_src = 1.0 / np.sqrt(deg_out)
    norm_dst = 1.0 / np.sqrt(deg_in)

    # partition-major h-table row of node s: (s%128 % 64)*392 + s//128,
    # in h_lo for s%128 < 64 else h_hi
    s_p = src % 128
    s_row = (s_p % 64) * NBLK + src // 128
    elo = s_p < 64

    # per-dst lo/hi in-degrees for the 2D balance
    dlo = np.bincount(dst[elo], minlength=N).astype(np.float32)
    dhi = deg_in - dlo
    grp_of, slot_of = _balance_groups(dlo, dhi)

    # normalized features, transposed + blocked for phase 1:
    # featT[p, (nb*4 + c)*128 + j] = Xn[nb*128 + j, c*128 + p]
    feat_n = (np.asarray(features, np.float32) * norm_src[:, None]).astype(bf16)
    xp = np.zeros((NPAD, F), bf16)
    xp[:N] = feat_n
    featT = np.ascontiguousarray(
        xp.reshape(NBLK, 128, 4, 128).transpose(3, 0, 2, 1)
    ).reshape(128, NBLK * 512)

    # edges -> (group, lo/hi, tile, lane)
    eg = grp_of[dst]
    cnt_lo = np.bincount(eg[elo], minlength=NG)
    cnt_hi = np.bincount(eg[~elo], minlength=NG)
    T_LO = int(np.ceil(cnt_lo.max() / 128))
    T_HI = int(np.ceil(cnt_hi.max() / 128))

    dloc = slot_of[dst].astype(np.float32)

    def layout(mask, cnts, L):
        r_e = s_row[mask]
        g_e = eg[mask]
        d_e = dloc[mask]
        # sort by (group, table row) so each tile's rows ascend in the table
        order = np.lexsort((r_e, g_e))
        g_s = g_e[order]
        starts = np.zeros(NG + 1, np.int64)
        np.cumsum(cnts, out=starts[1:])
        pos = np.arange(len(g_s)) - starts[g_s]
        slot = g_s * L + pos
        idx_pad = np.zeros(NG * L, np.int16)
        dl_pad = np.full(NG * L, 255.0, np.float32)
        idx_pad[slot] = r_e[order].astype(np.int16)
        dl_pad[slot] = d_e[order]
        return idx_pad.reshape(NG, L), dl_pad.reshape(NG, L)

    idx_lo, dl_lo = layout(elo, cnt_lo, T_LO * 128)
    idx_hi, dl_hi = layout(~elo, cnt_hi, T_HI * 128)

    # normdst per core: [NB*128] values in permuted slot order, pad 1.0
    nd = np.ones(NG * 128, np.float32)
    nd[grp_of * 128 + slot_of] = norm_dst

    iota = np.broadcast_to(np.arange(128, dtype=np.float32), (128, 128)).astype(bf16)
    wconv = np.ascontiguousarray(
        np.asarray(W_conv, np.float32).reshape(4, 128, H).transpose(1, 0, 2)
    ).reshape(128, 4 * H).astype(bf16)
    w2r = np.ascontiguousarray(
        np.asarray(W2, np.float32).reshape(2, 128, C).transpose(1, 0, 2)
    ).reshape(128, 2 * C).astype(bf16)

    in_maps = []
    for c in range(P):
        gsl = slice(c * NB, (c + 1) * NB)
        # dstloc per core: [128, NT] where tile column (b*T_T + t), lane p
        # holds the slot of edge (tile t of block b, lane p); lo tiles then
        # hi tiles within each block
        dl_c = np.concatenate(
            [dl_lo[gsl].reshape(NB, T_LO, 128),
             dl_hi[gsl].reshape(NB, T_HI, 128)], axis=1
        )  # [NB, T_T, 128]
        dstloc = np.ascontiguousarray(
            dl_c.reshape(NB * (T_LO + T_HI), 128).T).astype(bf16)
        ixlo = np.concatenate(
            [_pack_idx(idx_lo[g]) for g in range(c * NB, (c + 1) * NB)],
            axis=1)
        ixhi = np.concatenate(
            [_pack_idx(idx_hi[g]) for g in range(c * NB, (c + 1) * NB)],
            axis=1)
        in_maps.append({
            "featT": featT,
            "ixlo": np.ascontiguousarray(ixlo),
            "ixhi": np.ascontiguousarray(ixhi),
            "dstloc": dstloc,
            "normdst": np.ascontiguousarray(np.broadcast_to(
                nd[c * NPC:(c + 1) * NPC].astype(bf16), (128, NB * 128))),
            "iota": iota,
            "wconv": wconv,
            "w2": w2r,
            "ones1": np.ones((1, 128), np.float32).astype(bf16),
            "bconv": np.asarray(b_conv, np.float32).reshape(2, 128).T.copy(),
            "b2r": np.asarray(b2, np.float32).reshape(1, C).astype(bf16),
        })
    return T_LO, T_HI, grp_of, slot_of, in_maps


def kernel(features, W_conv, b_conv, W2, b2, src, dst):
    from concourse.bass_utils import run_bass_kernel_spmd

    T_LO, T_HI, grp_of, slot_of, in_maps = _prep(
        features, W_conv, b_conv, W2, b2, src, dst)
    key = (T_LO, T_HI)
    if key not in _cache:
        _cache[key] = _build_program(T_LO, T_HI)
    nc = _cache[key]
    res = run_bass_kernel_spmd(nc, in_maps, core_ids=list(range(P)),
                               trace=_trace, tmpdir=_trace_tmpdir)
    global _last_results
    _last_results = res
    rows = np.concatenate([res.results[c]["out"] for c in range(P)], axis=0)
    out = rows[grp_of * 128 + slot_of]
    return out.astype(np.float32)
